# revision 7
# baseline (speedup 1.0000x reference)
"""CfC (closed-form continuous-time) cell kernel for Trainium2, 8 NeuronCores.

Reference computation (B=8192, IN=256, H=512, all fp32):
    g     = sigmoid(x @ W_gx.T + b_gx + h @ W_gh.T + gate_b)        [B, H]
    f     = tanh(cat([x, h]) @ W_backbone.T + b_backbone)           [B, H]
    tau   = softplus(log_tau) + |g|          (g in (0,1) so |g| == g)
    decay = exp(-delta_t[:, None] * tau)
    out   = decay * h + (1 - decay) * f

Strategy: data-parallel over B (1024 rows per core), weights replicated.
Feature-major on device: activations ship as xh^T [768, B_shard] so the
contraction dim lands on SBUF partitions with no on-device transposes.

Matmul precision plan (v2):
  - GATE: fp8-e4m3 DoubleRow (2x PE rate).  Operands pre-scaled x*8, W*32;
    dequant + sigmoid half-angle fold into the ACT scale (1/512).
  - BACKBONE mixed: the x-part of the contraction (k-tiles 0-1) runs as ONE
    fp8 DoubleRow matmul REUSING the gate's fp8 activation panels (zero
    extra bytes); the h-part (k-tiles 2-5) stays fp16 with operands scaled
    h*8 / W*32 (exact powers of two) so the whole PSUM is 256*z and one
    ACT scale (1/256) dequants it.  Host-emulated rel err: 1.58e-2 (< 2e-2).
  This cuts PE work 15.5us -> 13.9us and input DMA 3.86MB -> 2.9MB.

Engine split: the 4 gate STT ops (t = (tg + 2sp+1) * (-dt/2)) and one dech
run on the otherwise-idle Pool (gpsimd) engine; -dt/2 ships as a [1, B]
row and is partition-broadcast on Pool, saving 254KB of wire at the ramp.
DVE keeps em/dech/combines.  ACT program order interleaves the two big
EXPs between backbone tanhs so no tanh waits on an exp it doesn't feed.

All input DMAs ride ONE HWDGE ring (sync) in exact consumption order, with
the gate weight/activation k-pair panels split into separate small tiles so
the first matmul fires ~700KB earlier than a whole-tile wait would allow.
Output ships fp16 per j-tile (last j per chunk) and is upcast on the host.
"""

from contextlib import ExitStack

import ml_dtypes
import numpy as np

import concourse.mybir as mybir
import concourse.tile as tile
from concourse import bacc
from concourse.bass_utils import run_bass_kernel_spmd

B, IN, H = 8192, 256, 512
NCORES = 8
BS = B // NCORES          # 1024 batch rows per core
KIN = IN + H              # 768 contraction dim
KT = KIN // 128           # 6 k-tiles
NP = KT // 2              # 3 k-pair panels (DoubleRow processes 2 k-tiles)
KH = 4                    # h-part k-tiles (fp16 backbone part)
NJ = H // 128             # 4 partition tiles per output matrix
NCHUNK = 512              # matmul moving free dim per PSUM bank
NCH = BS // NCHUNK        # 2 b-chunks per core

SX = 8.0                  # activation pre-scale (power of 2)
SW = 32.0                 # weight pre-scale (power of 2)
GDEQ = 1.0 / (SX * SW * 2.0)   # gate ACT scale: dequant + sigmoid half-angle
BDEQ = 1.0 / (SX * SW)         # backbone ACT scale: dequant only

F32 = mybir.dt.float32
FP16 = mybir.dt.float16
FP8 = mybir.dt.float8e4
AF = mybir.ActivationFunctionType
OP = mybir.AluOpType
PM = mybir.MatmulPerfMode

TRACE = False             # test.py flips this for profiled runs
LAST_RESULT = None        # BassKernelResults of the most recent run

_NC_CACHE = None


def _body(tc, ins, outP):
    nc = tc.nc
    with ExitStack() as ctx:
        singles = ctx.enter_context(tc.tile_pool(name="singles", bufs=1))
        decs = ctx.enter_context(tc.tile_pool(name="decs", bufs=4))
        work = ctx.enter_context(tc.tile_pool(name="work", bufs=3))
        psg = ctx.enter_context(tc.tile_pool(name="psg", bufs=2, space="PSUM"))
        psf = ctx.enter_context(tc.tile_pool(name="psf", bufs=2, space="PSUM"))

        # Persistent SBUF tiles.  Gate weight/activation streams are k-pair
        # panel tiles so each DMA completion unblocks exactly the matmuls
        # that need it (whole-tile waits would delay the first matmul ~1us).
        wg_t = [singles.tile([128, 2, H], FP8, tag=f"wg{g}", name=f"wg_t{g}")
                for g in range(NP)]
        xg_t = [[singles.tile([128, 2, NCHUNK], FP8, tag=f"xg{c}{g}",
                              name=f"xg_t{c}{g}")
                 for g in range(NP)] for c in range(NCH)]
        cst = singles.tile([128, 16], F32, tag="cst")
        ndtv = singles.tile([1, BS], FP16, tag="ndtv")
        ndt = singles.tile([128, BS], FP16, tag="ndt")
        wbx_t = singles.tile([128, 2, H], FP8, tag="wbx")
        wbh_t = singles.tile([128, KH, H], FP16, tag="wbh")
        xhh_t = [singles.tile([128, KH, NCHUNK], FP16, tag=f"xhh{c}",
                              name=f"xhh_t{c}")
                 for c in range(NCH)]

        # One sync HWDGE ring, FIFO in exact consumption order.
        def flat(t):
            return t.rearrange("p k c -> p (k c)")

        nc.sync.dma_start(out=flat(wg_t[0]), in_=ins["wg0"])
        nc.sync.dma_start(out=flat(xg_t[0][0]), in_=ins["xg00"])
        nc.sync.dma_start(out=flat(wg_t[1]), in_=ins["wg1"])
        nc.sync.dma_start(out=flat(xg_t[0][1]), in_=ins["xg01"])
        nc.sync.dma_start(out=flat(wg_t[2]), in_=ins["wg2"])
        nc.sync.dma_start(out=flat(xg_t[0][2]), in_=ins["xg02"])
        for g in range(NP):
            nc.sync.dma_start(out=flat(xg_t[1][g]), in_=ins[f"xg1{g}"])
        nc.sync.dma_start(out=cst, in_=ins["cst"])
        nc.sync.dma_start(out=ndtv, in_=ins["ndtv"])
        nc.sync.dma_start(out=flat(wbx_t), in_=ins["wbx"])
        nc.sync.dma_start(out=flat(wbh_t), in_=ins["wbh"])
        nc.sync.dma_start(out=flat(xhh_t[0]), in_=ins["xhh0"])
        nc.sync.dma_start(out=flat(xhh_t[1]), in_=ins["xhh1"])

        # Pool: broadcast -dt/2 row to all 128 partitions (saves 254KB of
        # ramp-critical wire vs shipping the broadcast from the host).
        nc.gpsimd.partition_broadcast(ndt, ndtv[:, :])

        gb = lambda j: cst[:, j:j + 1]           # (b_gx+gate_b)/2
        bbias = lambda j: cst[:, 4 + j:5 + j]    # b_backbone
        c2 = lambda j: cst[:, 8 + j:9 + j]       # 2*softplus(log_tau)+1
        zbias = cst[:, 12:13]                    # 0.0

        # --- Gate phase: DoubleRow fp8 matmuls + tanh; STT on Pool ---
        tp = {}
        for j in range(NJ):
            jsl = slice(j * 128, (j + 1) * 128)
            zg = psg.tile([128, BS], F32, tag="zg", name=f"zg_{j}")
            for c in range(NCH):
                bsl = slice(c * NCHUNK, (c + 1) * NCHUNK)
                for g in range(NP):
                    nc.tensor.matmul(
                        zg[:, bsl],
                        wg_t[g][:, :, jsl],
                        xg_t[c][g][:, :, :],
                        start=(g == 0),
                        stop=(g == NP - 1),
                        perf_mode=PM.DoubleRow,
                    )
            tg = work.tile([128, BS], FP16, tag="tg", name=f"tg_{j}")
            # tg = tanh((zg + 256*bg)/512) = tanh(zg_true/2 + bg/2)
            nc.scalar.activation(out=tg, in_=zg, func=AF.Tanh, bias=gb(j),
                                 scale=GDEQ)
            if j % 2 == 0:
                tp[j // 2] = work.tile([128, 2 * BS], FP16, tag="t",
                                       name=f"t_{j // 2}")
            # t = (tg + (2*softplus+1)) * (-dt/2)   [= -dt * (softplus + g)]
            # (Pool rejects STT with a per-partition scalar AP, so DVE.)
            nc.vector.scalar_tensor_tensor(
                out=tp[j // 2][:, (j % 2) * BS:(j % 2 + 1) * BS],
                in0=tg, scalar=c2(j), in1=ndt,
                op0=OP.add, op1=OP.mult,
            )

        # --- decay for pair 0; em/dech (j0, j1) ---
        dp01 = decs.tile([128, 2 * BS], FP16, tag="dec", name="dec_0")
        nc.scalar.activation(out=dp01, in_=tp[0], func=AF.Exp, bias=zbias)
        ep01 = decs.tile([128, 2 * BS], FP16, tag="em", name="em_0")
        nc.vector.tensor_scalar(out=ep01, in0=dp01, scalar1=-1.0, scalar2=1.0,
                                op0=OP.mult, op1=OP.add)
        dech = {}
        # dech1 rides the idle Pool engine (needed only at j1's combine,
        # ~3us later); dech0 is needed promptly so it stays on DVE.
        for jj, eng in ((0, nc.vector), (1, nc.gpsimd)):
            dh = decs.tile([128, BS], FP16, tag="dech", name=f"dech_{jj}")
            for c in range(NCH):
                csl = slice(c * NCHUNK, (c + 1) * NCHUNK)
                eng.tensor_mul(
                    out=dh[:, csl],
                    in0=dp01[:, jj * BS + c * NCHUNK:jj * BS + (c + 1) * NCHUNK],
                    in1=xhh_t[c][:, jj, :],
                )
            dech[jj] = dh

        # --- Backbone matmul group per (j, chunk): 1 DoubleRow fp8 (x-part,
        # reusing the gate's k-pair-0 panels) + KH fp16 (h-part). ---
        def bb_mms(dst, j, c):
            jsl = slice(j * 128, (j + 1) * 128)
            nc.tensor.matmul(
                dst, wbx_t[:, :, jsl], xg_t[c][0][:, :, :],
                start=True, stop=False, perf_mode=PM.DoubleRow,
            )
            for k in range(KH):
                nc.tensor.matmul(
                    dst, wbh_t[:, k, jsl], xhh_t[c][:, k, :],
                    start=False, stop=(k == KH - 1),
                )

        def combine(j, zf, csl, em_ap, name):
            w = csl.stop - csl.start
            f = work.tile([128, w], FP16, tag=f"f{w}", name=f"f_{name}")
            p = work.tile([128, w], FP16, tag=f"p{w}", name=f"p_{name}")
            o = work.tile([128, w], FP16, tag=f"o{w}", name=f"o_{name}")
            nc.scalar.activation(out=f, in_=zf[:, csl], func=AF.Tanh,
                                 bias=bbias(j), scale=BDEQ)
            nc.vector.tensor_mul(out=p, in0=f, in1=em_ap)
            nc.vector.tensor_add(out=o, in0=p, in1=dech[j][:, csl])
            nc.sync.dma_start(
                out=outP[:, j * BS + csl.start:j * BS + csl.stop], in_=o
            )

        # bb j0
        zf = psf.tile([128, BS], F32, tag="zf", name="zf_0")
        for c in range(NCH):
            bb_mms(zf[:, c * NCHUNK:(c + 1) * NCHUNK], 0, c)
        combine(0, zf, slice(0, BS), ep01[:, 0:BS], "0")

        # decay pair 1; em; dech2 on Pool (DVE is busy in this window),
        # dech3 on DVE.
        dp23 = decs.tile([128, 2 * BS], FP16, tag="dec", name="dec_1")
        nc.scalar.activation(out=dp23, in_=tp[1], func=AF.Exp, bias=zbias)
        ep23 = decs.tile([128, 2 * BS], FP16, tag="em", name="em_1")
        nc.vector.tensor_scalar(out=ep23, in0=dp23, scalar1=-1.0, scalar2=1.0,
                                op0=OP.mult, op1=OP.add)
        for jj, eng in ((2, nc.gpsimd), (3, nc.vector)):
            dh = decs.tile([128, BS], FP16, tag="dech", name=f"dech_{jj}")
            for c in range(NCH):
                csl = slice(c * NCHUNK, (c + 1) * NCHUNK)
                eng.tensor_mul(
                    out=dh[:, csl],
                    in0=dp23[:, (jj - 2) * BS + c * NCHUNK:
                             (jj - 2) * BS + (c + 1) * NCHUNK],
                    in1=xhh_t[c][:, jj, :],
                )
            dech[jj] = dh

        # bb j1, j2
        for j in (1, 2):
            zf = psf.tile([128, BS], F32, tag="zf", name=f"zf_{j}")
            for c in range(NCH):
                bb_mms(zf[:, c * NCHUNK:(c + 1) * NCHUNK], j, c)
            em_ap = ep01[:, BS:2 * BS] if j == 1 else ep23[:, 0:BS]
            combine(j, zf, slice(0, BS), em_ap, f"{j}")

        # Last j-tile: per-chunk PSUM tiles (reusing the now-idle gate psg
        # tag ring) so the chunk-0 epilogue starts before chunk-1's matmuls
        # finish and the final chain spans one chunk only.
        j = NJ - 1
        for c in range(NCH):
            zt = psg.tile([128, BS], F32, tag="zg", name=f"zf3_{c}")
            bb_mms(zt[:, 0:NCHUNK], j, c)
            csl = slice(c * NCHUNK, (c + 1) * NCHUNK)
            f = work.tile([128, NCHUNK], FP16, tag="fq", name=f"f_3_{c}")
            p = work.tile([128, NCHUNK], FP16, tag="pq", name=f"p_3_{c}")
            o = work.tile([128, NCHUNK], FP16, tag="oq", name=f"o_3_{c}")
            nc.scalar.activation(out=f, in_=zt[:, 0:NCHUNK], func=AF.Tanh,
                                 bias=bbias(j), scale=BDEQ)
            nc.vector.tensor_mul(out=p, in0=f,
                                 in1=ep23[:, BS + c * NCHUNK:BS + (c + 1) * NCHUNK])
            nc.vector.tensor_add(out=o, in0=p, in1=dech[j][:, csl])
            nc.sync.dma_start(
                out=outP[:, j * BS + csl.start:j * BS + csl.stop], in_=o
            )


def build_nc():
    nc = bacc.Bacc(
        "TRN2",
        target_bir_lowering=False,
        debug=False,
        enable_asserts=False,
        num_devices=NCORES,
    )
    ins = {}
    for g in range(NP):
        ins[f"wg{g}"] = nc.dram_tensor(f"wg{g}", [128, 2 * H], FP8,
                                       kind="ExternalInput").ap()
        for c in range(NCH):
            ins[f"xg{c}{g}"] = nc.dram_tensor(
                f"xg{c}{g}", [128, 2 * NCHUNK], FP8, kind="ExternalInput"
            ).ap()
    ins["cst"] = nc.dram_tensor("cst", [128, 16], F32, kind="ExternalInput").ap()
    ins["ndtv"] = nc.dram_tensor("ndtv", [1, BS], FP16, kind="ExternalInput").ap()
    ins["wbx"] = nc.dram_tensor("wbx", [128, 2 * H], FP8, kind="ExternalInput").ap()
    ins["wbh"] = nc.dram_tensor("wbh", [128, KH * H], FP16,
                                kind="ExternalInput").ap()
    for c in range(NCH):
        ins[f"xhh{c}"] = nc.dram_tensor(
            f"xhh{c}", [128, KH * NCHUNK], FP16, kind="ExternalInput"
        ).ap()
    outP = nc.dram_tensor("outP", [128, NJ * BS], FP16, kind="ExternalOutput").ap()
    with tile.TileContext(nc) as tc:
        _body(tc, ins, outP)
    nc.compile()
    return nc


def _get_nc():
    global _NC_CACHE
    if _NC_CACHE is None:
        _NC_CACHE = build_nc()
    return _NC_CACHE


def _pack_pmajor(a, kt):
    """[kt*128, C] -> [128, kt*C]: partition-major pack so each of the 128
    DMA rows is contiguous in DRAM."""
    c = a.shape[1]
    return np.ascontiguousarray(
        a.reshape(kt, 128, c).transpose(1, 0, 2).reshape(128, kt * c)
    )


def make_in_maps(x, h, delta_t, W_backbone, b_backbone, W_gx, b_gx, W_gh,
                 gate_b, log_tau):
    f32 = np.float32
    xh = np.concatenate(
        [np.asarray(x, f32), np.asarray(h, f32)], axis=1
    )                                                   # [B, 768]
    xhT = np.ascontiguousarray(xh.T)                    # [768, B] f32
    xg8 = np.asarray(xhT * SX, dtype=ml_dtypes.float8_e4m3)   # [768, B]
    # h-part fp16 UNSCALED (dech = decay*h reads these tiles directly);
    # the SX factor moves into the h-part weights instead (exact pow2).
    xhh16 = xhT[IN:, :].astype(np.float16)              # [512, B]

    WgT = np.concatenate(
        [np.asarray(W_gx, f32), np.asarray(W_gh, f32)], axis=1
    ).T                                                 # [768, H]
    wg8 = np.asarray(WgT * SW, dtype=ml_dtypes.float8_e4m3)
    WbT = np.ascontiguousarray(np.asarray(W_backbone, f32).T)  # [768, H]
    wbx8 = _pack_pmajor(
        np.asarray(WbT[:IN, :] * SW, dtype=ml_dtypes.float8_e4m3), 2
    )
    wbh16 = _pack_pmajor((WbT[IN:, :] * (SW * SX)).astype(np.float16), KH)
    wg_p = [_pack_pmajor(wg8[g * 256:(g + 1) * 256, :], 2) for g in range(NP)]

    sp2 = 2.0 * np.log1p(np.exp(np.asarray(log_tau, f32))) + 1.0
    cstv = np.stack(
        [
            (np.asarray(b_gx, f32) + np.asarray(gate_b, f32)) * 0.5,
            np.asarray(b_backbone, f32),
            sp2,
        ]
    )                                                   # [3, H]
    cstP = np.zeros((128, 16), f32)
    cstP[:, 0:12] = (
        cstv.reshape(3, NJ, 128).transpose(2, 0, 1).reshape(128, 12)
    )
    ndt2 = (np.asarray(delta_t, f32) * -0.5).astype(np.float16)   # [B]

    in_maps = []
    for core in range(NCORES):
        sl = slice(core * BS, (core + 1) * BS)
        m = {}
        for g in range(NP):
            m[f"wg{g}"] = wg_p[g]
            for c in range(NCH):
                cs = slice(core * BS + c * NCHUNK, core * BS + (c + 1) * NCHUNK)
                m[f"xg{c}{g}"] = _pack_pmajor(
                    np.ascontiguousarray(xg8[g * 256:(g + 1) * 256, cs]), 2
                )
        m["cst"] = cstP
        m["ndtv"] = np.ascontiguousarray(ndt2[sl][None, :])
        m["wbx"] = wbx8
        m["wbh"] = wbh16
        for c in range(NCH):
            cs = slice(core * BS + c * NCHUNK, core * BS + (c + 1) * NCHUNK)
            m[f"xhh{c}"] = _pack_pmajor(
                np.ascontiguousarray(xhh16[:, cs]), KH
            )
        in_maps.append(m)
    return in_maps


def kernel(x, h, delta_t, W_backbone, b_backbone, W_gx, b_gx, W_gh, gate_b,
           log_tau):
    global LAST_RESULT
    in_maps = make_in_maps(x, h, delta_t, W_backbone, b_backbone, W_gx, b_gx,
                           W_gh, gate_b, log_tau)
    nc = _get_nc()
    res = run_bass_kernel_spmd(
        nc, in_maps, core_ids=list(range(NCORES)), trace=TRACE
    )
    LAST_RESULT = res
    # outP is [128, NJ*BS] partition-major; unpack to [H, BS] then gather.
    outs = []
    for r in res.results:
        op = r["outP"].reshape(128, NJ, BS).transpose(1, 0, 2).reshape(H, BS)
        outs.append(op)
    out = np.concatenate(outs, axis=1).T
    return np.ascontiguousarray(out).astype(np.float32)
